# revision 9
# baseline (speedup 1.0000x reference)
"""DenseCaps EM-routing kernel for 8 Trainium2 NeuronCores.

Strategy: data-parallel over batch (B=32 -> 4 per core, no collectives).
Per batch b (on device):
  - votes V[o,i,p,r] = sum_q poses[i,p,q] w[o,i,q,r] via PE matmuls, using a
    host-prebuilt block-diagonal poses operand (K = 32 i's x 4 q per block).
    Output layout "B2": two tiles [(o32h,r)=128 part, (p,i)=8192 free], fp16.
  - V^2 via ACT Square during PSUM readout.
  - B2 -> "F" layout ([i128 part, (p,o,r) free]) via PE block transposes
    (needed so the EM i-contraction has K = pure i).
  - 3 EM iterations:
      M-step: Sv/Sv2/S1 via PE cross-product matmuls (lhsT = r^T chunk
      [i,o], rhs = V|V^2 in F layout), diagonal extracted with a mask
      fused into the PSUM->SBUF copy + strided reduce.
      E-step (iters 0,1): Q = sum_d(-0.5 V^2/sig2 + mu V/sig2) via PE
      block-diag-over-o matmuls on B2 layout accumulating over (p,r) and
      both weight sets in one PSUM tile; logits = Q + g; softmax over o by
      PE-transposing logits to [i, o] and reducing along free dim.
  - outputs mu (capsule poses) and a_out.
"""

import os
import numpy as np

B, I, O, P4 = 32, 2048, 64, 4
D = 16
EPS = 1e-7
NIT = 3
NCORES = 8
BL = B // NCORES          # batches per core
NBLK = I // 32            # 64 i-blocks of 32
NCH = I // 128            # 16 i-chunks of 128

F32 = "float32"
F16 = "float16"
BF16 = "bfloat16"


def _build_nc():
    import concourse.bass as bass
    import concourse.mybir as mybir
    import concourse.tile as tile
    from concourse import bacc

    dt = mybir.dt
    AX = mybir.AxisListType
    AL = mybir.AluOpType
    AF = mybir.ActivationFunctionType

    nc = bacc.Bacc("TRN2", target_bir_lowering=False, debug=False)

    # ---- DRAM I/O ----
    pd_d = nc.dram_tensor("pd", [BL, 128, NBLK * 128], dt.float16, kind="ExternalInput")
    wm_d = nc.dram_tensor("wm", [128, NBLK * 256], dt.float16, kind="ExternalInput")
    rt0_d = nc.dram_tensor("rt0", [BL, 128, NCH * 64], dt.float16, kind="ExternalInput")
    ait_d = nc.dram_tensor("ait", [BL, 128, NCH], dt.float32, kind="ExternalInput")
    betas_d = nc.dram_tensor("betas", [64, 4], dt.float32, kind="ExternalInput")
    mdo_d = nc.dram_tensor("mdo", [128, 32], dt.float16, kind="ExternalInput")
    dmask_d = nc.dram_tensor("dmask", [64, 64], dt.float16, kind="ExternalInput")
    id16_d = nc.dram_tensor("id16", [128, 128], dt.float16, kind="ExternalInput")
    id32_d = nc.dram_tensor("id32", [128, 128], dt.float32, kind="ExternalInput")
    ones_d = nc.dram_tensor("ones", [128, 4], dt.float16, kind="ExternalInput")
    actz_d = nc.dram_tensor("actz", [128, 2], dt.float32, kind="ExternalInput")
    muo_d = nc.dram_tensor("muo", [BL, 64, 16], dt.float32, kind="ExternalOutput")
    ao_d = nc.dram_tensor("ao", [BL, 64], dt.float32, kind="ExternalOutput")

    with tile.TileContext(nc) as tc:
        with tc.tile_pool(name="consts", bufs=1) as cp:
            wm = cp.tile([128, NBLK * 256], dt.float16)
            nc.sync.dma_start(out=wm, in_=wm_d.ap())
            mdo = cp.tile([128, 32], dt.float16)
            nc.sync.dma_start(out=mdo, in_=mdo_d.ap())
            dmask = cp.tile([64, 64], dt.float16)
            nc.sync.dma_start(out=dmask, in_=dmask_d.ap())
            id16 = cp.tile([128, 128], dt.float16)
            nc.sync.dma_start(out=id16, in_=id16_d.ap())
            id32 = cp.tile([128, 128], dt.float32)
            nc.sync.dma_start(out=id32, in_=id32_d.ap())
            ones = cp.tile([128, 4], dt.float16)
            nc.sync.dma_start(out=ones, in_=ones_d.ap())
            betas = cp.tile([64, 4], dt.float32)
            nc.sync.dma_start(out=betas, in_=betas_d.ap())
            actz = cp.tile([128, 2], dt.float32)
            nc.sync.dma_start(out=actz, in_=actz_d.ap())

            for b in range(BL):
                _body_one_b(
                    nc, tc, b,
                    pd_d, rt0_d, ait_d, muo_d, ao_d,
                    wm, mdo, dmask, id16, id32, ones, betas, actz,
                    dt, AX, AL, AF,
                )
    nc.compile()
    return nc


def _body_one_b(nc, tc, b, pd_d, rt0_d, ait_d, muo_d, ao_d,
                wm, mdo, dmask, id16, id32, ones, betas, actz, dt, AX, AL, AF):
    fp16 = dt.float16
    f32 = dt.float32

    with tc.tile_pool(name=f"b{b}", bufs=1) as bp:
        # ---------------- votes ----------------
        vb2 = [bp.tile([128, 8192], fp16, tag=f"vb2_{h}", name=f"vb2_{h}") for h in range(2)]
        v2b2 = [bp.tile([128, 8192], fp16, tag=f"v2b2_{h}", name=f"v2b2_{h}") for h in range(2)]

        with tc.tile_pool(name=f"pdp{b}", bufs=1) as pdp:
            pd_t = pdp.tile([128, NBLK * 128], fp16, tag="pd")
            nc.sync.dma_start(out=pd_t, in_=pd_d.ap()[b])

            with tc.tile_pool(name=f"vps{b}", bufs=3, space="PSUM") as vps:
                for g in range(16):          # groups of 4 blocks
                    for h in range(2):
                        ps = vps.tile([128, 512], f32, tag="vps")
                        for j in range(4):
                            blk = g * 4 + j
                            rhs = pd_t[:, blk * 128:(blk + 1) * 128].rearrange(
                                "k (i p) -> k p i", p=4)
                            lhsT = wm[:, blk * 256 + h * 128: blk * 256 + (h + 1) * 128]
                            nc.tensor.matmul(
                                ps[:, j * 128:(j + 1) * 128], lhsT, rhs,
                                start=True, stop=True)
                        # ps cols (j, p, i32); reorder to vb2 cols (p, i) in the copy
                        srcv = ps.rearrange("k (j p i) -> k p j i", j=4, p=4)
                        dst = vb2[h].rearrange(
                            "k (p j i) -> k p j i", p=4, i=32)[:, :, g * 4:(g + 1) * 4, :]
                        nc.any.tensor_copy(dst, srcv)
                        dst2 = v2b2[h].rearrange(
                            "k (p j i) -> k p j i", p=4, i=32)[:, :, g * 4:(g + 1) * 4, :]
                        nc.scalar.activation(dst2, srcv, AF.Square, bias=actz[:, 0:1])

        # ---------------- B2 -> F transposes ----------------
        vf = bp.tile([128, NCH * 1024], fp16, tag="vf")
        v2f = bp.tile([128, NCH * 1024], fp16, tag="v2f")
        with tc.tile_pool(name=f"tps{b}", bufs=4, space="PSUM") as tps:
            for c in range(NCH):
                pt = tps.tile([128, 1024], fp16, tag="t", name="pt")
                for p in range(4):
                    for h in range(2):
                        nc.tensor.transpose(
                            pt[:, p * 256 + h * 128: p * 256 + (h + 1) * 128],
                            vb2[h][:, p * 2048 + c * 128: p * 2048 + (c + 1) * 128],
                            id16)
                # pt cols (p, h, o32, r) -> F cols (p, r, o64); V^2 via ACT square
                srcv = pt.rearrange("k (p h o r) -> k p h o r", p=4, h=2, o=32)
                dstv = vf[:, c * 1024:(c + 1) * 1024].rearrange(
                    "k (p r h o) -> k p h o r", p=4, r=4, h=2)
                dstv2 = v2f[:, c * 1024:(c + 1) * 1024].rearrange(
                    "k (p r h o) -> k p h o r", p=4, r=4, h=2)
                for h in range(2):
                    nc.vector.tensor_copy(dstv[:, :, h], srcv[:, :, h])
                    nc.scalar.activation(dstv2[:, :, h], srcv[:, :, h],
                                         AF.Square, bias=actz[:, 0:1])

        # ---------------- EM iterations ----------------
        rt = bp.tile([128, NCH * 64], fp16, tag="rt0")
        nc.sync.dma_start(out=rt, in_=rt0_d.ap()[b])
        ait_t = bp.tile([128, NCH], f32, tag="ait")
        nc.sync.dma_start(out=ait_t, in_=ait_d.ap()[b])

        for t in range(NIT):
            lam = 1.0 + t
            # ---- M-step matmuls ----
            with tc.tile_pool(name=f"mps{b}_{t}", bufs=2, space="PSUM") as mps:
                psA = mps.tile([64, 1024], f32, tag="ms")
                psS = mps.tile([64, 2], f32, tag="s1")
                for c in range(NCH):
                    lhsT = rt[:, c * 64:(c + 1) * 64]
                    st, sp = (c == 0), (c == NCH - 1)
                    nc.tensor.matmul(psA[:, 0:512], lhsT,
                                     vf[:, c * 1024: c * 1024 + 512],
                                     start=st, stop=sp)
                    nc.tensor.matmul(psA[:, 512:1024], lhsT,
                                     vf[:, c * 1024 + 512: c * 1024 + 1024],
                                     start=st, stop=sp)
                    nc.tensor.matmul(psS[:, 0:1], lhsT, ones[:, 0:1],
                                     start=st, stop=sp)
                psB = mps.tile([64, 1024], f32, tag="ms")
                for c in range(NCH):
                    lhsT = rt[:, c * 64:(c + 1) * 64]
                    st, sp = (c == 0), (c == NCH - 1)
                    nc.tensor.matmul(psB[:, 0:512], lhsT,
                                     v2f[:, c * 1024: c * 1024 + 512],
                                     start=st, stop=sp)
                    nc.tensor.matmul(psB[:, 512:1024], lhsT,
                                     v2f[:, c * 1024 + 512: c * 1024 + 1024],
                                     start=st, stop=sp)

                # ---- masked diag extraction ----
                dmask_b = dmask[:].unsqueeze(1).broadcast_to([64, 16, 64])
                mskA = bp.tile([64, 1024], fp16, tag="mskA")
                nc.vector.scalar_tensor_tensor(
                    mskA.rearrange("k (x o) -> k x o", x=16),
                    psA.rearrange("k (x o) -> k x o", x=16),
                    1.0, dmask_b, AL.mult, AL.mult)
                mskB = bp.tile([64, 1024], fp16, tag="mskB")
                nc.vector.scalar_tensor_tensor(
                    mskB.rearrange("k (x o) -> k x o", x=16),
                    psB.rearrange("k (x o) -> k x o", x=16),
                    1.0, dmask_b, AL.mult, AL.mult)

                sv = bp.tile([64, 16], f32, tag="sv")
                nc.vector.tensor_reduce(
                    sv, mskA.rearrange("k (x o) -> k x o", x=16),
                    AX.X, AL.add)
                sv2 = bp.tile([64, 16], f32, tag="sv2")
                nc.vector.tensor_reduce(
                    sv2, mskB.rearrange("k (x o) -> k x o", x=16),
                    AX.X, AL.add)
                rs = bp.tile([64, 1], f32, tag="rs")
                nc.vector.tensor_scalar_add(rs, psS[:, 0:1], EPS)

            # ---- small per-(o,d) ops ----
            rsi = bp.tile([64, 1], f32, tag="rsi")
            nc.vector.reciprocal(rsi, rs)
            mu = bp.tile([64, 16], f32, tag="mu")
            nc.vector.tensor_scalar_mul(mu, sv, rsi)
            ex2 = bp.tile([64, 16], f32, tag="ex2")
            nc.vector.tensor_scalar_mul(ex2, sv2, rsi)
            mu2 = bp.tile([64, 16], f32, tag="mu2")
            nc.vector.tensor_mul(mu2, mu, mu)
            sig = bp.tile([64, 16], f32, tag="sig")
            nc.vector.scalar_tensor_tensor(sig, ex2, EPS, mu2, AL.add, AL.subtract)
            lg = bp.tile([64, 16], f32, tag="lg")
            nc.scalar.activation(lg, sig, AF.Ln, bias=actz[0:64, 0:1])
            L = bp.tile([64, 1], f32, tag="L")
            nc.vector.tensor_reduce(L, lg, AX.X, AL.add)
            # cost sum and a_out
            t1 = bp.tile([64, 1], f32, tag="t1")
            nc.vector.scalar_tensor_tensor(t1, L, 0.5, betas[:, 1:2], AL.mult, AL.add)
            csum = bp.tile([64, 1], f32, tag="csum")
            nc.vector.tensor_mul(csum, t1, rs)
            d1 = bp.tile([64, 1], f32, tag="d1")
            nc.vector.tensor_scalar(out=d1, in0=csum, scalar1=betas[:, 0:1],
                                    scalar2=None, op0=AL.subtract)
            eNZ = bp.tile([64, 1], f32, tag="eNZ")
            nc.scalar.activation(eNZ, d1, AF.Exp, bias=actz[0:64, 0:1], scale=lam)
            den = bp.tile([64, 1], f32, tag="den")
            nc.vector.tensor_scalar_add(den, eNZ, 1.0)
            aout = bp.tile([64, 1], f32, tag="aout")
            nc.vector.reciprocal(aout, den)

            if t == NIT - 1:
                nc.sync.dma_start(out=muo_d.ap()[b], in_=mu)
                nc.sync.dma_start(out=ao_d.ap()[b], in_=aout)
                continue

            # ---- E-step prep ----
            w2r = bp.tile([64, 16], f32, tag="w2r")
            nc.vector.reciprocal(w2r, sig)
            cc = bp.tile([64, 1], f32, tag="cc")
            junk = bp.tile([64, 16], f32, tag="junk")
            nc.vector.scalar_tensor_tensor(junk, mu2, 1.0, w2r, AL.mult, AL.mult,
                                           accum_out=cc)
            la = bp.tile([64, 1], f32, tag="la")
            nc.scalar.activation(la, aout, AF.Ln, bias=actz[0:64, 1:2])
            s2 = bp.tile([64, 1], f32, tag="s2")
            nc.vector.tensor_add(s2, cc, L)
            gv = bp.tile([64, 1], f32, tag="gv")
            nc.vector.scalar_tensor_tensor(gv, s2, -0.5, la, AL.mult, AL.add)

            # wsrc [64, 32] layout (r, m, p)
            wsrc = bp.tile([64, 32], f32, tag="wsrc")
            w2r_v = w2r.rearrange("o (p r) -> o r p", p=4)
            mu_v = mu.rearrange("o (p r) -> o r p", p=4)
            wsrc_m = wsrc.rearrange("o (r m p) -> o m r p", r=4, m=2)
            nc.vector.tensor_scalar_mul(wsrc_m[:, 0], w2r_v, -0.5)
            nc.vector.tensor_mul(wsrc_m[:, 1], mu_v, w2r_v)

            elh = []
            for h in range(2):
                wor_h = bp.tile([128, 8], f32, tag=f"wor{h}")
                nc.sync.dma_start(
                    out=wor_h,
                    in_=wsrc[32 * h: 32 * h + 32, :].rearrange(
                        "o (r x) -> o r x", x=8))
                e_h = bp.tile([128, 256], fp16, tag=f"elh{h}")
                in0 = wor_h[:].unsqueeze(2).broadcast_to([128, 8, 32])
                in1 = mdo[:].unsqueeze(1).broadcast_to([128, 8, 32])
                nc.vector.scalar_tensor_tensor(
                    e_h.rearrange("k (x o) -> k x o", x=8),
                    in0, 1.0, in1, AL.mult, AL.mult)
                elh.append(e_h)

            # ---- E-step matmuls: Q = sum_d (-0.5 V^2 + mu V)/sig2 ----
            with tc.tile_pool(name=f"eps{b}_{t}", bufs=1, space="PSUM") as eps:
                qps = eps.tile([64, 2048], f32, tag="q")
                for h in range(2):
                    for m in range(2):
                        rhs_t = v2b2 if m == 0 else vb2
                        for p in range(4):
                            lhsT = elh[h][:, (m * 4 + p) * 32:(m * 4 + p + 1) * 32]
                            st = (m == 0 and p == 0)
                            sp = (m == 1 and p == 3)
                            for n in range(4):
                                nc.tensor.matmul(
                                    qps[32 * h: 32 * h + 32, n * 512:(n + 1) * 512],
                                    lhsT,
                                    rhs_t[h][:, p * 2048 + n * 512: p * 2048 + (n + 1) * 512],
                                    start=st, stop=sp)

                lgt = bp.tile([64, 2048], f32, tag="lgt")
                for h in range(2):
                    nc.vector.tensor_scalar_add(
                        lgt[32 * h: 32 * h + 32, :],
                        qps[32 * h: 32 * h + 32, :],
                        gv[32 * h: 32 * h + 32, :])

                # ---- softmax over o (transpose to [i, o]) ----
                ltps = eps.tile([128, 1024], f32, tag="lt")
                for c in range(NCH):
                    nc.tensor.transpose(
                        ltps[:, c * 64:(c + 1) * 64],
                        lgt[:, c * 128:(c + 1) * 128],
                        id32[0:64, 0:64])
                eT = bp.tile([128, 1024], dt.bfloat16, tag="eT")
                nc.scalar.activation(eT, ltps, AF.Exp, bias=actz[:, 0:1])

            Z = bp.tile([128, NCH], f32, tag="Z")
            nc.vector.tensor_reduce(
                Z, eT.rearrange("k (c o) -> k c o", c=NCH), AX.X, AL.add)
            Zi = bp.tile([128, NCH], f32, tag="Zi")
            nc.vector.reciprocal(Zi, Z)
            za = bp.tile([128, NCH], f32, tag="za")
            nc.vector.tensor_mul(za, Zi, ait_t)
            rt_new = bp.tile([128, NCH * 64], fp16, tag=f"rt{t + 1}")
            nc.vector.tensor_mul(
                rt_new.rearrange("k (c o) -> k c o", c=NCH),
                eT.rearrange("k (c o) -> k c o", c=NCH),
                za[:].unsqueeze(2).broadcast_to([128, NCH, 64]))
            rt = rt_new


# ---------------- host-side input prep ----------------

def _prep_inputs(input_poses, input_activations, w, beta_a, beta_u):
    """Returns list of 8 per-core input dicts."""
    poses = np.asarray(input_poses, dtype=np.float32)
    ain = np.asarray(input_activations, dtype=np.float32)
    w = np.asarray(w, dtype=np.float32)
    beta_a = np.asarray(beta_a, dtype=np.float32)
    beta_u = np.asarray(beta_u, dtype=np.float32)

    # shared tensors
    wr = w.reshape(O, NBLK, 32, 4, 4)                       # o, blk, il, q, r
    wm = np.ascontiguousarray(wr.transpose(2, 3, 1, 0, 4))  # il, q, blk, o, r
    wm = wm.reshape(128, NBLK * 256).astype(np.float16)

    betas = np.zeros((64, 4), np.float32)
    betas[:, 0] = beta_a[0]
    betas[:, 1] = 16.0 * beta_u[0]

    mdo = np.zeros((128, 32), np.float16)
    for ol in range(32):
        mdo[ol * 4:(ol + 1) * 4, ol] = 1.0

    dmask = np.eye(64, dtype=np.float16)
    id16 = np.eye(128, dtype=np.float16)
    id32 = np.eye(128, dtype=np.float32)
    ones = np.ones((128, 4), np.float16)
    actz = np.zeros((128, 2), np.float32)
    actz[:, 1] = EPS

    in_maps = []
    for k in range(NCORES):
        pb = poses[k * BL:(k + 1) * BL]                     # [BL, I, 4, 4]
        ab = ain[k * BL:(k + 1) * BL]                       # [BL, I]

        src = pb.reshape(BL, NBLK, 32, 4, 4)                # b, blk, i2, p, q
        pdz = np.zeros((BL, 32, 4, NBLK, 32, 4), np.float16)  # b, il, q, blk, i2, p
        for i2 in range(32):
            # value poses[b, blk*32+i2, p, q] at [b, il==i2, q, blk, i2, p]
            pdz[:, i2, :, :, i2, :] = src[:, :, i2, :, :].transpose(0, 3, 1, 2)
        pd = pdz.reshape(BL, 128, NBLK * 128)

        a3 = ab.reshape(BL, NCH, 128)                        # b, c, ii
        ait = np.ascontiguousarray(a3.transpose(0, 2, 1))    # b, ii, c
        rt0 = np.repeat(
            (ait / 64.0)[..., None].astype(np.float16), 64, axis=-1
        ).reshape(BL, 128, NCH * 64)

        in_maps.append({
            "pd": pd,
            "wm": wm,
            "rt0": rt0,
            "ait": ait.astype(np.float32),
            "betas": betas,
            "mdo": mdo,
            "dmask": dmask,
            "id16": id16,
            "id32": id32,
            "ones": ones,
            "actz": actz,
        })
    return in_maps


_NC_CACHE = {}


def _get_nc():
    if "nc" not in _NC_CACHE:
        _NC_CACHE["nc"] = _build_nc()
    return _NC_CACHE["nc"]


def kernel(input_poses, input_activations, w, beta_a, beta_u):
    from concourse.bass_utils import run_bass_kernel_spmd

    nc = _get_nc()
    in_maps = _prep_inputs(input_poses, input_activations, w, beta_a, beta_u)
    res = run_bass_kernel_spmd(nc, in_maps, core_ids=list(range(NCORES)))
    mus = np.concatenate([r["muo"] for r in res.results], axis=0)   # [B, 64, 16]
    aos = np.concatenate([r["ao"] for r in res.results], axis=0)    # [B, 64]
    capsule_poses = mus.reshape(B, O, P4, P4).astype(np.float32)
    return capsule_poses, aos.astype(np.float32)


# revision 11
# speedup vs baseline: 982.7331x; 982.7331x over previous
"""DenseCaps EM-routing kernel for 8 Trainium2 NeuronCores.

Strategy: data-parallel over batch (B=32 -> 4 per core, no collectives).
Per batch b (on device):
  - votes V[o,i,p,r] = sum_q poses[i,p,q] w[o,i,q,r] via PE matmuls, using a
    host-prebuilt block-diagonal poses operand (K = 32 i's x 4 q per block).
    Output layout "B2": two tiles [(o32h,r)=128 part, (p,i)=8192 free], fp16.
  - V^2 via ACT Square during PSUM readout.
  - B2 -> "F" layout ([i128 part, (p,o,r) free]) via PE block transposes
    (needed so the EM i-contraction has K = pure i).
  - 3 EM iterations:
      M-step: Sv/Sv2/S1 via PE cross-product matmuls (lhsT = r^T chunk
      [i,o], rhs = V|V^2 in F layout), diagonal extracted with a mask
      fused into the PSUM->SBUF copy + strided reduce.
      E-step (iters 0,1): Q = sum_d(-0.5 V^2/sig2 + mu V/sig2) via PE
      block-diag-over-o matmuls on B2 layout accumulating over (p,r) and
      both weight sets in one PSUM tile; logits = Q + g; softmax over o by
      PE-transposing logits to [i, o] and reducing along free dim.
  - outputs mu (capsule poses) and a_out.
"""

import os
import numpy as np

B, I, O, P4 = 32, 2048, 64, 4
D = 16
EPS = 1e-7
NIT = 3
NCORES = 8
BL = B // NCORES          # batches per core
NBLK = I // 32            # 64 i-blocks of 32
NCH = I // 128            # 16 i-chunks of 128

F32 = "float32"
F16 = "float16"
BF16 = "bfloat16"


def _build_nc(repeats=1):
    import concourse.bass as bass
    import concourse.mybir as mybir
    import concourse.tile as tile
    from concourse import bacc

    dt = mybir.dt
    AX = mybir.AxisListType
    AL = mybir.AluOpType
    AF = mybir.ActivationFunctionType

    nc = bacc.Bacc("TRN2", target_bir_lowering=False, debug=False)

    # ---- DRAM I/O ----
    pd_d = nc.dram_tensor("pd", [BL, 128, NBLK * 128], dt.float16, kind="ExternalInput")
    wm_d = nc.dram_tensor("wm", [128, NBLK * 256], dt.float16, kind="ExternalInput")
    rt0_d = nc.dram_tensor("rt0", [BL, 128, NCH * 64], dt.float16, kind="ExternalInput")
    ait_d = nc.dram_tensor("ait", [BL, 128, NCH], dt.float32, kind="ExternalInput")
    betas_d = nc.dram_tensor("betas", [64, 4], dt.float32, kind="ExternalInput")
    mdo_d = nc.dram_tensor("mdo", [128, 32], dt.float16, kind="ExternalInput")
    dmask_d = nc.dram_tensor("dmask", [64, 64], dt.float16, kind="ExternalInput")
    id16_d = nc.dram_tensor("id16", [128, 128], dt.float16, kind="ExternalInput")
    id32_d = nc.dram_tensor("id32", [128, 128], dt.float32, kind="ExternalInput")
    ones_d = nc.dram_tensor("ones", [128, 4], dt.float16, kind="ExternalInput")
    actz_d = nc.dram_tensor("actz", [128, 2], dt.float32, kind="ExternalInput")
    muo_d = nc.dram_tensor("muo", [BL, 64, 16], dt.float32, kind="ExternalOutput")
    ao_d = nc.dram_tensor("ao", [BL, 64], dt.float32, kind="ExternalOutput")

    with tile.TileContext(nc) as tc:
        with tc.tile_pool(name="consts", bufs=1) as cp:
            wm = cp.tile([128, NBLK * 256], dt.float16)
            nc.sync.dma_start(out=wm, in_=wm_d.ap())
            mdo = cp.tile([128, 32], dt.float16)
            nc.sync.dma_start(out=mdo, in_=mdo_d.ap())
            dmask = cp.tile([64, 64], dt.float16)
            nc.sync.dma_start(out=dmask, in_=dmask_d.ap())
            id16 = cp.tile([128, 128], dt.float16)
            nc.sync.dma_start(out=id16, in_=id16_d.ap())
            id32 = cp.tile([128, 128], dt.float32)
            nc.sync.dma_start(out=id32, in_=id32_d.ap())
            ones = cp.tile([128, 4], dt.float16)
            nc.sync.dma_start(out=ones, in_=ones_d.ap())
            betas = cp.tile([64, 4], dt.float32)
            nc.sync.dma_start(out=betas, in_=betas_d.ap())
            actz = cp.tile([128, 2], dt.float32)
            nc.sync.dma_start(out=actz, in_=actz_d.ap())

            for _rep in range(repeats):
              for b in range(BL):
                _body_one_b(
                    nc, tc, b,
                    pd_d, rt0_d, ait_d, muo_d, ao_d,
                    wm, mdo, dmask, id16, id32, ones, betas, actz,
                    dt, AX, AL, AF,
                )
    nc.compile()
    return nc


def _body_one_b(nc, tc, b, pd_d, rt0_d, ait_d, muo_d, ao_d,
                wm, mdo, dmask, id16, id32, ones, betas, actz, dt, AX, AL, AF):
    fp16 = dt.float16
    f32 = dt.float32

    with tc.tile_pool(name=f"b{b}", bufs=1) as bp:
        # ---------------- votes ----------------
        vb2 = [bp.tile([128, 8192], fp16, tag=f"vb2_{h}", name=f"vb2_{h}") for h in range(2)]
        v2b2 = [bp.tile([128, 8192], fp16, tag=f"v2b2_{h}", name=f"v2b2_{h}") for h in range(2)]

        with tc.tile_pool(name=f"pdp{b}", bufs=1) as pdp:
            pd_t = pdp.tile([128, NBLK * 128], fp16, tag="pd")
            nc.sync.dma_start(out=pd_t, in_=pd_d.ap()[b])

            with tc.tile_pool(name=f"vps{b}", bufs=3, space="PSUM") as vps:
                for g in range(16):          # groups of 4 blocks
                    for h in range(2):
                        ps = vps.tile([128, 512], f32, tag="vps")
                        for j in range(4):
                            blk = g * 4 + j
                            rhs = pd_t[:, blk * 128:(blk + 1) * 128].rearrange(
                                "k (i p) -> k p i", p=4)
                            lhsT = wm[:, blk * 256 + h * 128: blk * 256 + (h + 1) * 128]
                            nc.tensor.matmul(
                                ps[:, j * 128:(j + 1) * 128], lhsT, rhs,
                                start=True, stop=True)
                        # ps cols (j, p, i32); reorder to vb2 cols (p, i) in the copy
                        srcv = ps.rearrange("k (j p i) -> k p j i", j=4, p=4)
                        dst = vb2[h].rearrange(
                            "k (p j i) -> k p j i", p=4, i=32)[:, :, g * 4:(g + 1) * 4, :]
                        nc.any.tensor_copy(dst, srcv)
                        dst2 = v2b2[h].rearrange(
                            "k (p j i) -> k p j i", p=4, i=32)[:, :, g * 4:(g + 1) * 4, :]
                        nc.scalar.activation(dst2, srcv, AF.Square, bias=actz[:, 0:1])

        # ---------------- B2 -> F transposes ----------------
        vf = bp.tile([128, NCH * 1024], fp16, tag="vf")
        v2f = bp.tile([128, NCH * 1024], fp16, tag="v2f")
        with tc.tile_pool(name=f"tps{b}", bufs=4, space="PSUM") as tps:
            for c in range(NCH):
                pt = tps.tile([128, 1024], fp16, tag="t", name="pt")
                for p in range(4):
                    for h in range(2):
                        nc.tensor.transpose(
                            pt[:, p * 256 + h * 128: p * 256 + (h + 1) * 128],
                            vb2[h][:, p * 2048 + c * 128: p * 2048 + (c + 1) * 128],
                            id16)
                # pt cols (p, h, o32, r) -> F cols (p, r, o64); V^2 via ACT square
                srcv = pt.rearrange("k (p h o r) -> k p h o r", p=4, h=2, o=32)
                dstv = vf[:, c * 1024:(c + 1) * 1024].rearrange(
                    "k (p r h o) -> k p h o r", p=4, r=4, h=2)
                dstv2 = v2f[:, c * 1024:(c + 1) * 1024].rearrange(
                    "k (p r h o) -> k p h o r", p=4, r=4, h=2)
                for h in range(2):
                    nc.vector.tensor_copy(dstv[:, :, h], srcv[:, :, h])
                    nc.scalar.activation(dstv2[:, :, h], srcv[:, :, h],
                                         AF.Square, bias=actz[:, 0:1])

        # ---------------- EM iterations ----------------
        rt = bp.tile([128, NCH * 64], fp16, tag="rt0")
        nc.sync.dma_start(out=rt, in_=rt0_d.ap()[b])
        ait_t = bp.tile([128, NCH], f32, tag="ait")
        nc.sync.dma_start(out=ait_t, in_=ait_d.ap()[b])

        for t in range(NIT):
            lam = 1.0 + t
            # ---- M-step matmuls ----
            with tc.tile_pool(name=f"mps{b}_{t}", bufs=1, space="PSUM") as mps:
                psAB = mps.tile([64, 2048], f32, tag="ms")
                psA = psAB[:, 0:1024]
                psB = psAB[:, 1024:2048]
                psS = mps.tile([64, 2], f32, tag="s1")
                for c in range(NCH):
                    lhsT = rt[:, c * 64:(c + 1) * 64]
                    st, sp = (c == 0), (c == NCH - 1)
                    nc.tensor.matmul(psA[:, 0:512], lhsT,
                                     vf[:, c * 1024: c * 1024 + 512],
                                     start=st, stop=sp)
                    nc.tensor.matmul(psA[:, 512:1024], lhsT,
                                     vf[:, c * 1024 + 512: c * 1024 + 1024],
                                     start=st, stop=sp)
                    nc.tensor.matmul(psB[:, 0:512], lhsT,
                                     v2f[:, c * 1024: c * 1024 + 512],
                                     start=st, stop=sp)
                    nc.tensor.matmul(psB[:, 512:1024], lhsT,
                                     v2f[:, c * 1024 + 512: c * 1024 + 1024],
                                     start=st, stop=sp)
                    nc.tensor.matmul(psS[:, 0:1], lhsT, ones[:, 0:1],
                                     start=st, stop=sp)

                # ---- masked diag extraction ----
                dmask_b = dmask[:].unsqueeze(1).broadcast_to([64, 16, 64])
                mskA = bp.tile([64, 1024], fp16, tag="mskA")
                nc.vector.scalar_tensor_tensor(
                    mskA.rearrange("k (x o) -> k x o", x=16),
                    psA.rearrange("k (x o) -> k x o", x=16),
                    1.0, dmask_b, AL.mult, AL.mult)
                mskB = bp.tile([64, 1024], fp16, tag="mskB")
                nc.vector.scalar_tensor_tensor(
                    mskB.rearrange("k (x o) -> k x o", x=16),
                    psB.rearrange("k (x o) -> k x o", x=16),
                    1.0, dmask_b, AL.mult, AL.mult)

                sv = bp.tile([64, 16], f32, tag="sv")
                nc.vector.tensor_reduce(
                    sv, mskA.rearrange("k (x o) -> k x o", x=16),
                    AX.X, AL.add)
                sv2 = bp.tile([64, 16], f32, tag="sv2")
                nc.vector.tensor_reduce(
                    sv2, mskB.rearrange("k (x o) -> k x o", x=16),
                    AX.X, AL.add)
                rs = bp.tile([64, 1], f32, tag="rs")
                nc.vector.tensor_scalar_add(rs, psS[:, 0:1], EPS)

            # ---- small per-(o,d) ops ----
            rsi = bp.tile([64, 1], f32, tag="rsi")
            nc.vector.reciprocal(rsi, rs)
            mu = bp.tile([64, 16], f32, tag="mu")
            nc.vector.tensor_scalar_mul(mu, sv, rsi)
            ex2 = bp.tile([64, 16], f32, tag="ex2")
            nc.vector.tensor_scalar_mul(ex2, sv2, rsi)
            mu2 = bp.tile([64, 16], f32, tag="mu2")
            nc.vector.tensor_mul(mu2, mu, mu)
            sig = bp.tile([64, 16], f32, tag="sig")
            nc.vector.scalar_tensor_tensor(sig, ex2, EPS, mu2, AL.add, AL.subtract)
            lg = bp.tile([64, 16], f32, tag="lg")
            nc.scalar.activation(lg, sig, AF.Ln, bias=actz[0:64, 0:1])
            L = bp.tile([64, 1], f32, tag="L")
            nc.vector.tensor_reduce(L, lg, AX.X, AL.add)
            # cost sum and a_out
            t1 = bp.tile([64, 1], f32, tag="t1")
            nc.vector.scalar_tensor_tensor(t1, L, 0.5, betas[:, 1:2], AL.mult, AL.add)
            csum = bp.tile([64, 1], f32, tag="csum")
            nc.vector.tensor_mul(csum, t1, rs)
            d1 = bp.tile([64, 1], f32, tag="d1")
            nc.vector.tensor_scalar(out=d1, in0=csum, scalar1=betas[:, 0:1],
                                    scalar2=None, op0=AL.subtract)
            eNZ = bp.tile([64, 1], f32, tag="eNZ")
            nc.scalar.activation(eNZ, d1, AF.Exp, bias=actz[0:64, 0:1], scale=lam)
            den = bp.tile([64, 1], f32, tag="den")
            nc.vector.tensor_scalar_add(den, eNZ, 1.0)
            aout = bp.tile([64, 1], f32, tag="aout")
            nc.vector.reciprocal(aout, den)

            if t == NIT - 1:
                nc.sync.dma_start(out=muo_d.ap()[b], in_=mu)
                nc.sync.dma_start(out=ao_d.ap()[b], in_=aout)
                continue

            # ---- E-step prep ----
            w2r = bp.tile([64, 16], f32, tag="w2r")
            nc.vector.reciprocal(w2r, sig)
            cc = bp.tile([64, 1], f32, tag="cc")
            junk = bp.tile([64, 16], f32, tag="junk")
            nc.vector.scalar_tensor_tensor(junk, mu2, 1.0, w2r, AL.mult, AL.mult,
                                           accum_out=cc)
            la = bp.tile([64, 1], f32, tag="la")
            nc.scalar.activation(la, aout, AF.Ln, bias=actz[0:64, 1:2])
            s2 = bp.tile([64, 1], f32, tag="s2")
            nc.vector.tensor_add(s2, cc, L)
            gv = bp.tile([64, 1], f32, tag="gv")
            nc.vector.scalar_tensor_tensor(gv, s2, -0.5, la, AL.mult, AL.add)

            # wsrc [64, 32] layout (r, m, p)
            wsrc = bp.tile([64, 32], f32, tag="wsrc")
            w2r_v = w2r.rearrange("o (p r) -> o r p", p=4)
            mu_v = mu.rearrange("o (p r) -> o r p", p=4)
            wsrc_m = wsrc.rearrange("o (r m p) -> o m r p", r=4, m=2)
            nc.vector.tensor_scalar_mul(wsrc_m[:, 0], w2r_v, -0.5)
            nc.vector.tensor_mul(wsrc_m[:, 1], mu_v, w2r_v)

            elh = []
            for h in range(2):
                wor_h = bp.tile([128, 8], f32, tag=f"wor{h}")
                nc.sync.dma_start(
                    out=wor_h,
                    in_=wsrc[32 * h: 32 * h + 32, :].rearrange(
                        "o (r x) -> o r x", x=8))
                e_h = bp.tile([128, 256], fp16, tag=f"elh{h}")
                in0 = wor_h[:].unsqueeze(2).broadcast_to([128, 8, 32])
                in1 = mdo[:].unsqueeze(1).broadcast_to([128, 8, 32])
                nc.vector.scalar_tensor_tensor(
                    e_h.rearrange("k (x o) -> k x o", x=8),
                    in0, 1.0, in1, AL.mult, AL.mult)
                elh.append(e_h)

            # ---- E-step matmuls: Q = sum_d (-0.5 V^2 + mu V)/sig2 ----
            with tc.tile_pool(name=f"eps{b}_{t}", bufs=1, space="PSUM") as eps:
                qps = eps.tile([64, 2048], f32, tag="q")
                for h in range(2):
                    for m in range(2):
                        rhs_t = v2b2 if m == 0 else vb2
                        for p in range(4):
                            lhsT = elh[h][:, (m * 4 + p) * 32:(m * 4 + p + 1) * 32]
                            st = (m == 0 and p == 0)
                            sp = (m == 1 and p == 3)
                            for n in range(4):
                                nc.tensor.matmul(
                                    qps[32 * h: 32 * h + 32, n * 512:(n + 1) * 512],
                                    lhsT,
                                    rhs_t[h][:, p * 2048 + n * 512: p * 2048 + (n + 1) * 512],
                                    start=st, stop=sp)

                lgt = bp.tile([64, 2048], f32, tag="lgt")
                for h in range(2):
                    nc.scalar.activation(
                        lgt[32 * h: 32 * h + 32, :],
                        qps[32 * h: 32 * h + 32, :],
                        AF.Identity, bias=gv[32 * h: 32 * h + 32, :])

                # ---- softmax over o (transpose to [i, o]) ----
                ltps = eps.tile([128, 1024], f32, tag="lt")
                for c in range(NCH):
                    nc.tensor.transpose(
                        ltps[:, c * 64:(c + 1) * 64],
                        lgt[:, c * 128:(c + 1) * 128],
                        id32[0:64, 0:64])
                eT = bp.tile([128, 1024], dt.bfloat16, tag="eT")
                nc.scalar.activation(eT, ltps, AF.Exp, bias=actz[:, 0:1])

            Z = bp.tile([128, NCH], f32, tag="Z")
            nc.vector.tensor_reduce(
                Z, eT.rearrange("k (c o) -> k c o", c=NCH), AX.X, AL.add)
            Zi = bp.tile([128, NCH], f32, tag="Zi")
            nc.vector.reciprocal(Zi, Z)
            za = bp.tile([128, NCH], f32, tag="za")
            nc.vector.tensor_mul(za, Zi, ait_t)
            rt_new = bp.tile([128, NCH * 64], fp16, tag=f"rt{t + 1}")
            nc.vector.tensor_mul(
                rt_new.rearrange("k (c o) -> k c o", c=NCH),
                eT.rearrange("k (c o) -> k c o", c=NCH),
                za[:].unsqueeze(2).broadcast_to([128, NCH, 64]))
            rt = rt_new


# ---------------- host-side input prep ----------------

def _prep_inputs(input_poses, input_activations, w, beta_a, beta_u):
    """Returns list of 8 per-core input dicts."""
    poses = np.asarray(input_poses, dtype=np.float32)
    ain = np.asarray(input_activations, dtype=np.float32)
    w = np.asarray(w, dtype=np.float32)
    beta_a = np.asarray(beta_a, dtype=np.float32)
    beta_u = np.asarray(beta_u, dtype=np.float32)

    # shared tensors
    wr = w.reshape(O, NBLK, 32, 4, 4)                       # o, blk, il, q, r
    wm = np.ascontiguousarray(wr.transpose(2, 3, 1, 0, 4))  # il, q, blk, o, r
    wm = wm.reshape(128, NBLK * 256).astype(np.float16)

    betas = np.zeros((64, 4), np.float32)
    betas[:, 0] = beta_a[0]
    betas[:, 1] = 16.0 * beta_u[0]

    mdo = np.zeros((128, 32), np.float16)
    for ol in range(32):
        mdo[ol * 4:(ol + 1) * 4, ol] = 1.0

    dmask = np.eye(64, dtype=np.float16)
    id16 = np.eye(128, dtype=np.float16)
    id32 = np.eye(128, dtype=np.float32)
    ones = np.ones((128, 4), np.float16)
    actz = np.zeros((128, 2), np.float32)
    actz[:, 1] = EPS

    in_maps = []
    for k in range(NCORES):
        pb = poses[k * BL:(k + 1) * BL]                     # [BL, I, 4, 4]
        ab = ain[k * BL:(k + 1) * BL]                       # [BL, I]

        src = pb.reshape(BL, NBLK, 32, 4, 4)                # b, blk, i2, p, q
        pdz = np.zeros((BL, 32, 4, NBLK, 32, 4), np.float16)  # b, il, q, blk, i2, p
        for i2 in range(32):
            # value poses[b, blk*32+i2, p, q] at [b, il==i2, q, blk, i2, p]
            pdz[:, i2, :, :, i2, :] = src[:, :, i2, :, :].transpose(0, 3, 1, 2)
        pd = pdz.reshape(BL, 128, NBLK * 128)

        a3 = ab.reshape(BL, NCH, 128)                        # b, c, ii
        ait = np.ascontiguousarray(a3.transpose(0, 2, 1))    # b, ii, c
        rt0 = np.repeat(
            (ait / 64.0)[..., None].astype(np.float16), 64, axis=-1
        ).reshape(BL, 128, NCH * 64)

        in_maps.append({
            "pd": pd,
            "wm": wm,
            "rt0": rt0,
            "ait": ait.astype(np.float32),
            "betas": betas,
            "mdo": mdo,
            "dmask": dmask,
            "id16": id16,
            "id32": id32,
            "ones": ones,
            "actz": actz,
        })
    return in_maps


_NC_CACHE = {}


def _get_nc():
    if "nc" not in _NC_CACHE:
        _NC_CACHE["nc"] = _build_nc()
    return _NC_CACHE["nc"]


def kernel(input_poses, input_activations, w, beta_a, beta_u):
    from concourse.bass_utils import run_bass_kernel_spmd

    nc = _get_nc()
    in_maps = _prep_inputs(input_poses, input_activations, w, beta_a, beta_u)
    res = run_bass_kernel_spmd(nc, in_maps, core_ids=list(range(NCORES)))
    mus = np.concatenate([r["muo"] for r in res.results], axis=0)   # [B, 64, 16]
    aos = np.concatenate([r["ao"] for r in res.results], axis=0)    # [B, 64]
    capsule_poses = mus.reshape(B, O, P4, P4).astype(np.float32)
    return capsule_poses, aos.astype(np.float32)
